# revision 35
# baseline (speedup 1.0000x reference)
"""Trainium2 Bass kernel for nn_DA_conv: per-sample dynamic depthwise 3x3 conv
(+LeakyReLU) followed by a 1x1 pointwise conv, with the 3x3 kernels produced by
a small per-sample MLP.

Strategy (8 NeuronCores, pure batch data-parallel, 2 samples per core):
  - The tiny kernel-generating MLP runs on the host; the device receives the
    per-(sample,channel) tap scalars and prebuilt 128x128 block-diagonal fp16
    tap matrices in a single small DMA.
  - SBUF partition p = (sample s = p//64, channel c = p%64); fp16 feature map
    resident in SBUF with zero-padded borders (row stride 132, halo rows).
  - Depthwise 3x3 conv split spatially across engines, 4-row windows (512 px
    = one PSUM bank) as the unit:
      * PE windows (22): 9 PSUM-accumulating 128-partition block-diagonal
        fp16 matmuls per window (both samples in one matmul = 1 cycle/row).
      * DVE pair-units (5, interleaved ~1:2 with PE units to track the x-DMA
        arrival frontier): 8-row in-place MAC chains (scalar_tensor_tensor)
        into f32 SBUF accumulators. Pairs split into two partials - products
        on GPSIMD + ScalarE, 7 MACs on VectorE, merge-add on GPSIMD - so all
        four engines contribute. The last DVE unit stays single-accumulator
        and lrelu-evacuates itself (max(x, 0.1x)) so the drain never queues
        on ScalarE.
  - LeakyReLU evacuation (PSUM/SBUF acc -> fp16 D) on ScalarE via Prelu; the
    Prelu activation table is preloaded at t=0 via a dummy op so the first
    evacuation does not stall on the 1.3us table load.
  - 1x1 conv = one 128x128 block-diag fp16 matmul per window; bias added
    during the PSUM->SBUF evac on ScalarE via Prelu(alpha=1) with a bias AP.
  - A single early warm-up matmul starts the PE p-state ramp clock during
    the DMA prologue, so real matmuls run at full rate from the start.
  - Output staged as fp16 (host upcasts); the final pair unit runs as 2-row
    windows with evacuations split across ScalarE/VectorE and fine-grained
    output DMAs to minimize the drain chain.
"""

import os
import sys

sys.path.insert(0, "/opt/trn_rl_repo")

from contextlib import ExitStack

import numpy as np

import concourse.bacc as bacc
import concourse.bass as bass
import concourse.mybir as mybir
import concourse.tile as tile

S = 2            # samples per core
C = 64           # channels
H = W = 128      # spatial
KK = 3           # conv kernel size
NCORES = 8
RS = W + 4       # padded row stride (132 fp16 -> 264B; pads at cols 0,129..131)
RP = H + 2       # padded row count (top/bottom halo)
XFREE = RP * RS
WR = 4           # image rows per window
NWIN = H // WR   # 32 windows
NU = NWIN // 2   # 16 pair units
WPX = WR * W     # 512 px per window = one PSUM bank of f32

f32 = mybir.dt.float32
fp16 = mybir.dt.float16
i32 = mybir.dt.int32

PRELU = mybir.ActivationFunctionType.Prelu
TAPS = [(di, dj) for di in range(KK) for dj in range(KK)]  # t = di*3 + dj

# Pair units owned by the DVE MAC path, interleaved ~1:2 with PE units so
# both engines consume windows near the x-DMA arrival frontier; the last
# units stay on PE so the trailing 1x1 matmuls never wait on VectorE.
N_DVE_UNITS = int(os.environ.get("DA_NDVE", "5"))
DVE_UNITS = [1, 4, 7, 10, 13] if N_DVE_UNITS == 5 else \
    [1, 4, 5, 7, 10, 13][:N_DVE_UNITS]
MERGE_UNITS = (set(DVE_UNITS[:-1])
               if os.environ.get("DA_MERGE", "1") == "1" else set())
DVE_EXTRA = [22] if os.environ.get("DA_EXTRA", "0") == "1" else []
SLIVER = os.environ.get("DA_SLIVER", "0") == "1"
DVE_WINDOWS = sorted(
    [w for u in DVE_UNITS for w in (2 * u, 2 * u + 1)] + DVE_EXTRA
)


def build_program() -> bass.Bass:
    nc = bacc.Bacc("TRN2", target_bir_lowering=False, debug=False)

    x_d = nc.dram_tensor("x", [S * C, H * W], fp16, kind="ExternalInput").ap()
    # host-computed fp16 small weights: [bc (1) | kcols (9) | diag (9*128)]
    kd_d = nc.dram_tensor("kd", [S * C, 10 + KK * KK * S * C], fp16,
                          kind="ExternalInput").ap()
    # block-diagonal duplicated 1x1 weights, fp16
    wcb_d = nc.dram_tensor("wcb", [S * C, S * C], fp16, kind="ExternalInput").ap()
    out_d = nc.dram_tensor("out", [S * C, H * W], fp16, kind="ExternalOutput").ap()

    with tile.TileContext(nc) as tc, ExitStack() as ctx:
        _body(ctx, tc, x_d, kd_d, wcb_d, out_d)
    nc.compile()
    return nc


def _body(ctx, tc, x_d, kd_d, wcb_d, out_d):
    nc = tc.nc
    const = ctx.enter_context(tc.tile_pool(name="const", bufs=1))
    xpool = ctx.enter_context(tc.tile_pool(name="xs", bufs=1))
    dpool = ctx.enter_context(tc.tile_pool(name="dd", bufs=1))
    pdw = ctx.enter_context(tc.tile_pool(name="pdw", bufs=2, space="PSUM"))
    po = ctx.enter_context(tc.tile_pool(name="po", bufs=2, space="PSUM"))

    # ---------------- weight loads ----------------
    kd = const.tile([2 * C, 10 + KK * KK * 2 * C], fp16)
    diag = kd[:, 10:]
    kbf = const.tile([2 * C, 10], f32)
    bc2 = kbf[:, 0:1]
    kcols = kbf[:, 1:10]

    # ---------------- resident padded feature map ----------------
    xs = xpool.tile([2 * C, XFREE], fp16)
    # top halo row + row-1 left pad, bottom halo row; then the interior pad
    # columns: right pads (129..131) of row r are contiguous with the left pad
    # (col 0) of row r+1, so one strided memset covers all of them.
    nc.gpsimd.memset(xs[:, 0 : RS + 1], 0.0)
    nc.gpsimd.memset(xs[:, (RP - 1) * RS : RP * RS], 0.0)
    pads = xs[:, W + 1 : W + 1 + (H + 1) * RS].rearrange("p (r w) -> p r w", w=RS)
    nc.gpsimd.memset(pads[:, :, 0:4], 0.0)

    # x DMA in chunks; first chunks small so compute can start early
    def dma_chunk(r0, cr):
        src = x_d[:, r0 * W : (r0 + cr) * W].rearrange("p (r w) -> p r w", w=W)
        o = (r0 + 1) * RS + 1
        dst = xs[:, o : o + cr * RS].rearrange("p (r w) -> p r w", w=RS)[:, :, 0:W]
        nc.sync.dma_start(dst, src)

    dma_chunk(0, 6)
    nc.sync.dma_start(kd[:, :], kd_d)
    # per-partition scalars must be f32 for DVE/GPSIMD ops
    nc.vector.tensor_copy(kbf[:, :], kd[:, 0:10])
    dma_chunk(6, 10)
    dma_chunk(16, 8)
    dma_chunk(24, 8)
    wcb = const.tile([2 * C, 2 * C], fp16)
    nc.sync.dma_start(wcb[:, :], wcb_d)
    r0 = 32
    for cr in [12, 12, 12, 12, 16, 16, 16]:
        dma_chunk(r0, cr)
        r0 += cr

    # ---------------- main loop ----------------
    xrows = xs[:, :].rearrange("p (r w) -> p r w", w=RS)
    D = dpool.tile([2 * C, H * W], fp16)    # lrelu(dw), 1x1 rhs
    D2 = dpool.tile([2 * C, H * W], fp16)   # 1x1 out + bias, DMA staging
    n_dve_w = len(DVE_WINDOWS)
    accv = (dpool.tile([2 * C, n_dve_w * WPX + WPX // 2, ], f32, name="accv")
            if n_dve_w else None)
    accbp = ctx.enter_context(tc.tile_pool(name="accb", bufs=2))
    dve_w_idx = {w: i for i, w in enumerate(DVE_WINDOWS)}

    def win_ap(w, di, dj):
        # rhs/in0 window for tap (di, dj) over image rows 4w..4w+3
        return xrows[:, WR * w + di : WR * w + di + WR, dj : dj + W]

    def dw_pe(w, dst):
        for t, (di, dj) in enumerate(TAPS):
            nc.tensor.matmul(
                dst,
                lhsT=diag[:, t * 128 : (t + 1) * 128],
                rhs=win_ap(w, di, dj),
                start=(t == 0), stop=(t == KK * KK - 1),
                skip_group_check=True,
            )

    def dw_pe_pair(u):
        P2 = pdw.tile([128, 2 * WPX], f32, tag="pdw")
        for half in range(2):
            dw_pe(2 * u + half, P2[:, half * WPX : (half + 1) * WPX])
        nc.scalar.activation(
            D[:, 2 * u * WPX : (2 * u + 2) * WPX], P2[:, :], PRELU, alpha=0.1
        )

    def dw_pe_half(w0, q):
        # 2-row (256 px) window: quarter q of the final pair
        HPX = WPX // 2
        P1 = pdw.tile([128, HPX], f32, tag="pdw", name=f"ph{q}")
        for t, (di, dj) in enumerate(TAPS):
            nc.tensor.matmul(
                P1[:, :],
                lhsT=diag[:, t * 128 : (t + 1) * 128],
                rhs=xrows[:, WR * w0 + 2 * q + di : WR * w0 + 2 * q + di + 2,
                          dj : dj + W],
                start=(t == 0), stop=(t == KK * KK - 1),
                skip_group_check=True,
            )
        nc.scalar.activation(
            D[:, w0 * WPX + q * HPX : w0 * WPX + (q + 1) * HPX], P1[:, :],
            PRELU, alpha=0.1,
        )

    def dw_pe_single(w, evac_dve=False):
        P1 = pdw.tile([128, WPX], f32, tag="pdw", name=f"ps{w}")
        dw_pe(w, P1[:, :])
        dd = D[:, w * WPX : (w + 1) * WPX]
        if evac_dve:
            # lrelu(x) = max(x, 0.1x) as a single DVE op
            nc.vector.scalar_tensor_tensor(
                dd, P1[:, :], 0.1, P1[:, :],
                op0=mybir.AluOpType.mult, op1=mybir.AluOpType.max,
            )
        else:
            nc.scalar.activation(dd, P1[:, :], PRELU, alpha=0.1)

    def dw_dve_pair(u, merge=False):
        # 8-row (1024 px) in-place DVE MAC chains over both windows of the
        # unit at once; tap-0 product on GPSIMD. With merge=True, taps split
        # into two partial accumulators (second product on GPSIMD, merge-add
        # on GPSIMD) so VectorE only runs 7 of the 9 taps.
        def pwin(di, dj):
            return xrows[:, 8 * u + di : 8 * u + di + 8, dj : dj + W]

        base = dve_w_idx[2 * u] * WPX
        a = accv[:, base : base + 2 * WPX]
        if merge:
            # two partials: tap-0 product on GPSIMD, tap-4 product on ScalarE,
            # 3+4 MACs on VectorE, merge-add on GPSIMD
            nc.gpsimd.tensor_scalar_mul(a, pwin(0, 0), kcols[:, 0:1])
            b = accbp.tile([2 * C, 2 * WPX], f32, tag="accB", name=f"ab{u}")
            nc.scalar.mul(b[:, :], pwin(*TAPS[4]), kcols[:, 4:5])
            for t in (1, 2, 3):
                nc.vector.scalar_tensor_tensor(
                    a, pwin(*TAPS[t]), kcols[:, t : t + 1], a,
                    op0=mybir.AluOpType.mult, op1=mybir.AluOpType.add,
                )
            for t in (5, 6, 7, 8):
                nc.vector.scalar_tensor_tensor(
                    b[:, :], pwin(*TAPS[t]), kcols[:, t : t + 1], b[:, :],
                    op0=mybir.AluOpType.mult, op1=mybir.AluOpType.add,
                )
            nc.gpsimd.tensor_tensor(a, a, b[:, :], mybir.AluOpType.add)
        else:
            nc.gpsimd.tensor_scalar_mul(a, pwin(0, 0), kcols[:, 0:1])
            for t in range(1, KK * KK):
                nc.vector.scalar_tensor_tensor(
                    a, pwin(*TAPS[t]), kcols[:, t : t + 1], a,
                    op0=mybir.AluOpType.mult, op1=mybir.AluOpType.add,
                )
        dd = D[:, 2 * u * WPX : (2 * u + 2) * WPX]
        if u == DVE_UNITS[-1]:
            # acc is SBUF: lrelu(x) = max(x, 0.1x) directly on the (by now
            # otherwise idle) VectorE, keeping the drain off ScalarE's queue
            nc.vector.scalar_tensor_tensor(
                dd, a, 0.1, a, op0=mybir.AluOpType.mult,
                op1=mybir.AluOpType.max,
            )
        else:
            nc.scalar.activation(dd, a, PRELU, alpha=0.1)

    def dw_dve_sliver(w):
        # first 2 rows (256 px) of window w on DVE, into the spare acc slot
        HPX = WPX // 2
        a = accv[:, len(DVE_WINDOWS) * WPX : len(DVE_WINDOWS) * WPX + HPX]
        win = lambda di, dj: xrows[:, WR * w + di : WR * w + di + 2, dj : dj + W]
        nc.gpsimd.tensor_scalar_mul(a, win(0, 0), kcols[:, 0:1])
        for t in range(1, KK * KK):
            nc.vector.scalar_tensor_tensor(
                a, win(*TAPS[t]), kcols[:, t : t + 1], a,
                op0=mybir.AluOpType.mult, op1=mybir.AluOpType.add,
            )
        nc.scalar.activation(
            D[:, w * WPX : w * WPX + HPX], a, PRELU, alpha=0.1
        )

    def dw_pe_rest(w):
        # rows 2..3 (256 px) of window w on PE
        HPX = WPX // 2
        P1 = pdw.tile([128, HPX], f32, tag="pdw", name=f"pr{w}")
        for t, (di, dj) in enumerate(TAPS):
            nc.tensor.matmul(
                P1[:, :],
                lhsT=diag[:, t * 128 : (t + 1) * 128],
                rhs=xrows[:, WR * w + 2 + di : WR * w + 2 + di + 2, dj : dj + W],
                start=(t == 0), stop=(t == KK * KK - 1),
                skip_group_check=True,
            )
        nc.scalar.activation(
            D[:, w * WPX + HPX : (w + 1) * WPX], P1[:, :], PRELU, alpha=0.1
        )

    def dw_dve_single(w):
        # one 4-row window on DVE (single accumulator)
        a = accv[:, dve_w_idx[w] * WPX : (dve_w_idx[w] + 1) * WPX]
        nc.gpsimd.tensor_scalar_mul(a, win_ap(w, 0, 0), kcols[:, 0:1])
        for t in range(1, KK * KK):
            nc.vector.scalar_tensor_tensor(
                a, win_ap(w, *TAPS[t]), kcols[:, t : t + 1], a,
                op0=mybir.AluOpType.mult, op1=mybir.AluOpType.add,
            )
        nc.scalar.activation(D[:, w * WPX : (w + 1) * WPX], a, PRELU, alpha=0.1)

    assert all(2 * (w // 2) in dve_w_idx and 2 * (w // 2) + 1 in dve_w_idx
               for w in DVE_WINDOWS if w not in DVE_EXTRA), \
        "DVE pair windows must come in full pairs"

    def conv1x1_pair(u, evac_dve=False):
        O2 = po.tile([128, 2 * WPX], f32, tag="po")
        for half in range(2):
            w = 2 * u + half
            nc.tensor.matmul(
                O2[:, half * WPX : (half + 1) * WPX],
                lhsT=wcb[:, :],
                rhs=D[:, w * WPX : (w + 1) * WPX],
                start=True, stop=True,
                skip_group_check=True,
            )
        d2 = D2[:, 2 * u * WPX : (2 * u + 2) * WPX]
        if evac_dve:
            nc.vector.tensor_scalar_add(d2, O2[:, :], bc2)
        else:
            nc.scalar.activation(d2, O2[:, :], PRELU,
                                 bias=bc2, scale=1.0, alpha=1.0)

    def conv1x1_single(w, evac_dve=False):
        O1 = po.tile([128, WPX], f32, tag="po", name=f"po{w}")
        nc.tensor.matmul(
            O1[:, :], lhsT=wcb[:, :], rhs=D[:, w * WPX : (w + 1) * WPX],
            start=True, stop=True, skip_group_check=True,
        )
        d2 = D2[:, w * WPX : (w + 1) * WPX]
        if evac_dve:
            nc.vector.tensor_scalar_add(d2, O1[:, :], bc2)
        else:
            nc.scalar.activation(d2, O1[:, :], PRELU,
                                 bias=bc2, scale=1.0, alpha=1.0)

    # warm-up matmuls: ramp the PE p-state while the x DMA streams in.
    # Reads the (memset) top halo row; result discarded.
    warm = po.tile([128, 128], f32, tag="po", name="warm")
    for _ in range(int(os.environ.get("DA_WARM", "1"))):
        nc.tensor.matmul(warm[:, :], lhsT=xs[:, 0:128], rhs=xs[:, 0:128],
                         start=True, stop=True, skip_group_check=True)
    # touch ScalarE once so its Prelu table load happens during the prologue
    # instead of stalling the first PSUM evacuation
    wsc = const.tile([128, 1], f32)
    nc.scalar.activation(wsc[:, :], warm[:, 0:1], PRELU, alpha=0.1)

    for u in range(NU):
        if u == 0:
            dw_pe_single(0)
            dw_pe_single(1)
        elif u == NU - 1:
            for q in range(4):
                dw_pe_half(2 * u, q)
        elif 2 * u in dve_w_idx:
            dw_dve_pair(u, merge=(u in MERGE_UNITS))
        elif 2 * u in [w - (w % 2) for w in DVE_EXTRA]:
            dw_dve_single(2 * u)
            dw_pe_single(2 * u + 1)
        elif u == 11 and SLIVER:
            dw_pe_single(22)
            dw_dve_sliver(23)
            dw_pe_rest(23)
        else:
            dw_pe_pair(u)
        if u == NU - 1:
            HPX = WPX // 2
            OQA = po.tile([128, WPX], f32, tag="po", name="oqa")
            OQB = po.tile([128, WPX], f32, tag="po", name="oqb")
            for q in range(4):
                dst = (OQA, OQB)[q // 2]
                nc.tensor.matmul(
                    dst[:, (q % 2) * HPX : (q % 2 + 1) * HPX], lhsT=wcb[:, :],
                    rhs=D[:, 2 * u * WPX + q * HPX : 2 * u * WPX + (q + 1) * HPX],
                    start=True, stop=True, skip_group_check=True,
                )
            nc.scalar.activation(
                D2[:, 2 * u * WPX : (2 * u + 1) * WPX], OQA[:, :], PRELU,
                bias=bc2, scale=1.0, alpha=1.0,
            )
            nc.vector.tensor_scalar_add(
                D2[:, (2 * u + 1) * WPX : (2 * u + 2) * WPX], OQB[:, :], bc2,
            )
            o = (2 * u - 2) * WPX
            nc.sync.dma_start(out_d[:, o : o + 2 * WPX], D2[:, o : o + 2 * WPX])
            o += 2 * WPX
            nc.sync.dma_start(out_d[:, o : o + 2 * WPX], D2[:, o : o + 2 * WPX])
        else:
            conv1x1_pair(u, evac_dve=(u == NU - 2))
            if u % 2 == 1:
                o = (u - 1) * 2 * WPX
                nc.sync.dma_start(out_d[:, o : o + 4 * WPX], D2[:, o : o + 4 * WPX])


# ---------------------------------------------------------------------------
# host-side entry point
# ---------------------------------------------------------------------------

_PROGRAM_CACHE: dict[str, bass.Bass] = {}


def _get_program() -> bass.Bass:
    if "p" not in _PROGRAM_CACHE:
        _PROGRAM_CACHE["p"] = build_program()
    return _PROGRAM_CACHE["p"]


def _host_prep(inputs: dict):
    x = np.asarray(inputs["x"], dtype=np.float32)
    d = np.asarray(inputs["d"], dtype=np.float32)
    Wk1 = np.asarray(inputs["Wk1"], dtype=np.float32)
    Wk2 = np.asarray(inputs["Wk2"], dtype=np.float32)
    Wc = np.asarray(inputs["Wc"], dtype=np.float32)
    bc = np.asarray(inputs["bc"], dtype=np.float32)

    # kernel-generating MLP on the host (tiny): kern[b, c, t]
    hid = d @ Wk1.T
    hid = np.where(hid >= 0, hid, 0.1 * hid)
    kern = (hid @ Wk2.T).reshape(-1, C, KK * KK)
    bc2 = np.concatenate([bc, bc]).reshape(2 * C, 1)

    wcb = np.zeros((2 * C, 2 * C), dtype=np.float32)
    wcb[0:C, 0:C] = Wc.T
    wcb[C:, C:] = Wc.T
    wcb = wcb.astype(np.float16)

    xcast = x.astype(np.float16)

    in_maps = []
    for i in range(NCORES):
        xi = np.ascontiguousarray(xcast[S * i : S * (i + 1)].reshape(S * C, H * W))
        kc = kern[S * i : S * (i + 1)].reshape(2 * C, KK * KK)  # (s*64+c, t)
        dg = np.zeros((2 * C, KK * KK, 2 * C), dtype=np.float32)
        idx = np.arange(2 * C)
        dg[idx, :, idx] = kc
        kd = np.concatenate(
            [bc2, kc, dg.reshape(2 * C, KK * KK * 2 * C)], axis=1
        ).astype(np.float16)
        kd = np.ascontiguousarray(kd)
        in_maps.append({"x": xi, "kd": kd, "wcb": wcb})
    return in_maps


def run_on_hw(inputs: dict, **kwargs):
    """Run the SPMD kernel on 8 NeuronCores; returns (output, BassKernelResults)."""
    from concourse.bass_utils import run_bass_kernel_spmd

    nc = _get_program()
    in_maps = _host_prep(inputs)
    res = run_bass_kernel_spmd(nc, in_maps, core_ids=list(range(NCORES)), **kwargs)
    outs = res.results
    B = S * NCORES
    out = np.empty((B, C, H, W), dtype=np.float32)
    for i in range(NCORES):
        out[S * i : S * (i + 1)] = (
            outs[i]["out"].view(np.float16).astype(np.float32).reshape(S, C, H, W)
        )
    return out, res


def kernel(**inputs) -> np.ndarray:
    out, _ = run_on_hw(inputs)
    return out


if __name__ == "__main__":
    nc = build_program()
    print("program built OK")


# revision 41
# speedup vs baseline: 1.0025x; 1.0025x over previous
"""Trainium2 Bass kernel for nn_DA_conv: per-sample dynamic depthwise 3x3 conv
(+LeakyReLU) followed by a 1x1 pointwise conv, with the 3x3 kernels produced by
a small per-sample MLP.

Strategy (8 NeuronCores, pure batch data-parallel, 2 samples per core):
  - The tiny kernel-generating MLP runs on the host; the device receives the
    per-(sample,channel) tap scalars and prebuilt 128x128 block-diagonal fp16
    tap matrices in a single small DMA.
  - SBUF partition p = (sample s = p//64, channel c = p%64); fp16 feature map
    resident in SBUF with zero-padded borders (row stride 132, halo rows).
  - Depthwise 3x3 conv split spatially across engines, 4-row windows (512 px
    = one PSUM bank) as the unit:
      * PE windows (22): 9 PSUM-accumulating 128-partition block-diagonal
        fp16 matmuls per window (both samples in one matmul = 1 cycle/row).
      * DVE pair-units (5, interleaved ~1:2 with PE units to track the x-DMA
        arrival frontier): 8-row in-place MAC chains (scalar_tensor_tensor)
        into f32 SBUF accumulators. Pairs split into two partials - products
        on GPSIMD + ScalarE, 7 MACs on VectorE, merge-add on GPSIMD - so all
        four engines contribute. The last DVE unit stays single-accumulator
        and lrelu-evacuates itself (max(x, 0.1x)) so the drain never queues
        on ScalarE.
  - LeakyReLU evacuation (PSUM/SBUF acc -> fp16 D) on ScalarE via Prelu; the
    Prelu activation table is preloaded at t=0 via a dummy op so the first
    evacuation does not stall on the 1.3us table load.
  - 1x1 conv = one 128x128 block-diag fp16 matmul per window; bias added
    during the PSUM->SBUF evac on ScalarE via Prelu(alpha=1) with a bias AP.
  - A single early warm-up matmul starts the PE p-state ramp clock during
    the DMA prologue, so real matmuls run at full rate from the start.
  - Output staged as fp16 (host upcasts); the final pair unit runs as 2-row
    windows with evacuations split across ScalarE/VectorE and fine-grained
    output DMAs to minimize the drain chain.
"""

import os
import sys

sys.path.insert(0, "/opt/trn_rl_repo")

from contextlib import ExitStack

import numpy as np

import concourse.bacc as bacc
import concourse.bass as bass
import concourse.mybir as mybir
import concourse.tile as tile

S = 2            # samples per core
C = 64           # channels
H = W = 128      # spatial
KK = 3           # conv kernel size
NCORES = 8
RS = W + 4       # padded row stride (132 fp16 -> 264B; pads at cols 0,129..131)
RP = H + 2       # padded row count (top/bottom halo)
XFREE = RP * RS
WR = 4           # image rows per window
NWIN = H // WR   # 32 windows
NU = NWIN // 2   # 16 pair units
WPX = WR * W     # 512 px per window = one PSUM bank of f32

f32 = mybir.dt.float32
fp16 = mybir.dt.float16
i32 = mybir.dt.int32

PRELU = mybir.ActivationFunctionType.Prelu
TAPS = [(di, dj) for di in range(KK) for dj in range(KK)]  # t = di*3 + dj

# Pair units owned by the DVE MAC path, interleaved ~1:2 with PE units so
# both engines consume windows near the x-DMA arrival frontier; the last
# units stay on PE so the trailing 1x1 matmuls never wait on VectorE.
N_DVE_UNITS = int(os.environ.get("DA_NDVE", "5"))
_DVE_POS = {
    "a": [1, 4, 7, 10, 13],
    "b": [1, 4, 6, 9, 13],
    "c": [2, 5, 8, 11, 13],
    "d": [1, 3, 6, 9, 12],
}
DVE_UNITS = (_DVE_POS[os.environ.get("DA_POS", "a")] if N_DVE_UNITS == 5
             else [1, 4, 5, 7, 10, 13][:N_DVE_UNITS])
MERGE_UNITS = (set(DVE_UNITS[:-1])
               if os.environ.get("DA_MERGE", "1") == "1" else set())
DVE_EXTRA = [22] if os.environ.get("DA_EXTRA", "0") == "1" else []
SLIVER = os.environ.get("DA_SLIVER", "0") == "1"
# final pair processed in tapering pieces (row offset, nrows) over its 8 rows
TAIL_PIECES = [(0, 2), (2, 2), (4, 2), (6, 2)]
DVE_WINDOWS = sorted(
    [w for u in DVE_UNITS for w in (2 * u, 2 * u + 1)] + DVE_EXTRA
)


def build_program() -> bass.Bass:
    nc = bacc.Bacc("TRN2", target_bir_lowering=False, debug=False)

    x_d = nc.dram_tensor("x", [S * C, H * W], fp16, kind="ExternalInput").ap()
    # host-computed fp16 small weights: [bc (1) | kcols (9) | diag (9*128)]
    kd_d = nc.dram_tensor("kd", [S * C, 10 + KK * KK * S * C], fp16,
                          kind="ExternalInput").ap()
    # block-diagonal duplicated 1x1 weights, fp16
    wcb_d = nc.dram_tensor("wcb", [S * C, S * C], fp16, kind="ExternalInput").ap()
    out_d = nc.dram_tensor("out", [S * C, H * W], fp16, kind="ExternalOutput").ap()

    with tile.TileContext(nc) as tc, ExitStack() as ctx:
        _body(ctx, tc, x_d, kd_d, wcb_d, out_d)
    nc.compile()
    return nc


def _body(ctx, tc, x_d, kd_d, wcb_d, out_d):
    nc = tc.nc
    const = ctx.enter_context(tc.tile_pool(name="const", bufs=1))
    xpool = ctx.enter_context(tc.tile_pool(name="xs", bufs=1))
    dpool = ctx.enter_context(tc.tile_pool(name="dd", bufs=1))
    pdw = ctx.enter_context(tc.tile_pool(name="pdw", bufs=2, space="PSUM"))
    po = ctx.enter_context(tc.tile_pool(name="po", bufs=2, space="PSUM"))

    # ---------------- weight loads ----------------
    kd = const.tile([2 * C, 10 + KK * KK * 2 * C], fp16)
    diag = kd[:, 10:]
    kbf = const.tile([2 * C, 10], f32)
    bc2 = kbf[:, 0:1]
    kcols = kbf[:, 1:10]

    # ---------------- resident padded feature map ----------------
    xs = xpool.tile([2 * C, XFREE], fp16)
    # top halo row + row-1 left pad, bottom halo row; then the interior pad
    # columns: right pads (129..131) of row r are contiguous with the left pad
    # (col 0) of row r+1, so one strided memset covers all of them.
    nc.gpsimd.memset(xs[:, 0 : RS + 1], 0.0)
    nc.gpsimd.memset(xs[:, (RP - 1) * RS : RP * RS], 0.0)
    pads = xs[:, W + 1 : W + 1 + (H + 1) * RS].rearrange("p (r w) -> p r w", w=RS)
    nc.gpsimd.memset(pads[:, :, 0:4], 0.0)

    # x DMA in chunks; first chunks small so compute can start early
    def dma_chunk(r0, cr):
        src = x_d[:, r0 * W : (r0 + cr) * W].rearrange("p (r w) -> p r w", w=W)
        o = (r0 + 1) * RS + 1
        dst = xs[:, o : o + cr * RS].rearrange("p (r w) -> p r w", w=RS)[:, :, 0:W]
        nc.sync.dma_start(dst, src)

    dma_chunk(0, 5)
    nc.sync.dma_start(kd[:, :], kd_d)
    # per-partition scalars must be f32 for DVE/GPSIMD ops
    nc.vector.tensor_copy(kbf[:, :], kd[:, 0:10])
    dma_chunk(5, 11)
    dma_chunk(16, 8)
    dma_chunk(24, 8)
    wcb = const.tile([2 * C, 2 * C], fp16)
    nc.sync.dma_start(wcb[:, :], wcb_d)
    r0 = 32
    for cr in [12, 12, 12, 12, 16, 16, 16]:
        dma_chunk(r0, cr)
        r0 += cr

    # ---------------- main loop ----------------
    xrows = xs[:, :].rearrange("p (r w) -> p r w", w=RS)
    D = dpool.tile([2 * C, H * W], fp16)    # lrelu(dw), 1x1 rhs
    D2 = dpool.tile([2 * C, H * W], fp16)   # 1x1 out + bias, DMA staging
    n_dve_w = len(DVE_WINDOWS)
    accv = (dpool.tile([2 * C, n_dve_w * WPX + WPX // 2, ], f32, name="accv")
            if n_dve_w else None)
    accbp = ctx.enter_context(tc.tile_pool(name="accb", bufs=2))
    dve_w_idx = {w: i for i, w in enumerate(DVE_WINDOWS)}

    def win_ap(w, di, dj):
        # rhs/in0 window for tap (di, dj) over image rows 4w..4w+3
        return xrows[:, WR * w + di : WR * w + di + WR, dj : dj + W]

    def dw_pe(w, dst):
        for t, (di, dj) in enumerate(TAPS):
            nc.tensor.matmul(
                dst,
                lhsT=diag[:, t * 128 : (t + 1) * 128],
                rhs=win_ap(w, di, dj),
                start=(t == 0), stop=(t == KK * KK - 1),
                skip_group_check=True,
            )

    def dw_pe_pair(u):
        P2 = pdw.tile([128, 2 * WPX], f32, tag="pdw")
        for half in range(2):
            dw_pe(2 * u + half, P2[:, half * WPX : (half + 1) * WPX])
        nc.scalar.activation(
            D[:, 2 * u * WPX : (2 * u + 2) * WPX], P2[:, :], PRELU, alpha=0.1
        )

    def dw_pe_half(w0, q, ro, nr):
        # nr-row (nr*128 px) piece of the final pair, at row offset ro
        px = nr * W
        P1 = pdw.tile([128, px], f32, tag="pdw", name=f"ph{q}")
        for t, (di, dj) in enumerate(TAPS):
            nc.tensor.matmul(
                P1[:, :],
                lhsT=diag[:, t * 128 : (t + 1) * 128],
                rhs=xrows[:, WR * w0 + ro + di : WR * w0 + ro + di + nr,
                          dj : dj + W],
                start=(t == 0), stop=(t == KK * KK - 1),
                skip_group_check=True,
            )
        o = w0 * WPX + ro * W
        nc.scalar.activation(D[:, o : o + px], P1[:, :], PRELU, alpha=0.1)

    def dw_pe_single(w, evac_dve=False):
        P1 = pdw.tile([128, WPX], f32, tag="pdw", name=f"ps{w}")
        dw_pe(w, P1[:, :])
        dd = D[:, w * WPX : (w + 1) * WPX]
        if evac_dve:
            # lrelu(x) = max(x, 0.1x) as a single DVE op
            nc.vector.scalar_tensor_tensor(
                dd, P1[:, :], 0.1, P1[:, :],
                op0=mybir.AluOpType.mult, op1=mybir.AluOpType.max,
            )
        else:
            nc.scalar.activation(dd, P1[:, :], PRELU, alpha=0.1)

    def dw_dve_pair(u, merge=False):
        # 8-row (1024 px) in-place DVE MAC chains over both windows of the
        # unit at once; tap-0 product on GPSIMD. With merge=True, taps split
        # into two partial accumulators (second product on GPSIMD, merge-add
        # on GPSIMD) so VectorE only runs 7 of the 9 taps.
        def pwin(di, dj):
            return xrows[:, 8 * u + di : 8 * u + di + 8, dj : dj + W]

        base = dve_w_idx[2 * u] * WPX
        a = accv[:, base : base + 2 * WPX]
        if merge:
            # two partials: tap-0 product on GPSIMD, tap-4 product on ScalarE,
            # 3+4 MACs on VectorE, merge-add on GPSIMD
            nc.gpsimd.tensor_scalar_mul(a, pwin(0, 0), kcols[:, 0:1])
            b = accbp.tile([2 * C, 2 * WPX], f32, tag="accB", name=f"ab{u}")
            nc.scalar.mul(b[:, :], pwin(*TAPS[4]), kcols[:, 4:5])
            for t in (1, 2, 3):
                nc.vector.scalar_tensor_tensor(
                    a, pwin(*TAPS[t]), kcols[:, t : t + 1], a,
                    op0=mybir.AluOpType.mult, op1=mybir.AluOpType.add,
                )
            for t in (5, 6, 7, 8):
                nc.vector.scalar_tensor_tensor(
                    b[:, :], pwin(*TAPS[t]), kcols[:, t : t + 1], b[:, :],
                    op0=mybir.AluOpType.mult, op1=mybir.AluOpType.add,
                )
            nc.gpsimd.tensor_tensor(a, a, b[:, :], mybir.AluOpType.add)
        else:
            nc.gpsimd.tensor_scalar_mul(a, pwin(0, 0), kcols[:, 0:1])
            for t in range(1, KK * KK):
                nc.vector.scalar_tensor_tensor(
                    a, pwin(*TAPS[t]), kcols[:, t : t + 1], a,
                    op0=mybir.AluOpType.mult, op1=mybir.AluOpType.add,
                )
        dd = D[:, 2 * u * WPX : (2 * u + 2) * WPX]
        if u == DVE_UNITS[-1]:
            # acc is SBUF: lrelu(x) = max(x, 0.1x) directly on the (by now
            # otherwise idle) VectorE, keeping the drain off ScalarE's queue
            nc.vector.scalar_tensor_tensor(
                dd, a, 0.1, a, op0=mybir.AluOpType.mult,
                op1=mybir.AluOpType.max,
            )
        else:
            nc.scalar.activation(dd, a, PRELU, alpha=0.1)

    def dw_dve_sliver(w):
        # first 2 rows (256 px) of window w on DVE, into the spare acc slot
        HPX = WPX // 2
        a = accv[:, len(DVE_WINDOWS) * WPX : len(DVE_WINDOWS) * WPX + HPX]
        win = lambda di, dj: xrows[:, WR * w + di : WR * w + di + 2, dj : dj + W]
        nc.gpsimd.tensor_scalar_mul(a, win(0, 0), kcols[:, 0:1])
        for t in range(1, KK * KK):
            nc.vector.scalar_tensor_tensor(
                a, win(*TAPS[t]), kcols[:, t : t + 1], a,
                op0=mybir.AluOpType.mult, op1=mybir.AluOpType.add,
            )
        nc.scalar.activation(
            D[:, w * WPX : w * WPX + HPX], a, PRELU, alpha=0.1
        )

    def dw_pe_rest(w):
        # rows 2..3 (256 px) of window w on PE
        HPX = WPX // 2
        P1 = pdw.tile([128, HPX], f32, tag="pdw", name=f"pr{w}")
        for t, (di, dj) in enumerate(TAPS):
            nc.tensor.matmul(
                P1[:, :],
                lhsT=diag[:, t * 128 : (t + 1) * 128],
                rhs=xrows[:, WR * w + 2 + di : WR * w + 2 + di + 2, dj : dj + W],
                start=(t == 0), stop=(t == KK * KK - 1),
                skip_group_check=True,
            )
        nc.scalar.activation(
            D[:, w * WPX + HPX : (w + 1) * WPX], P1[:, :], PRELU, alpha=0.1
        )

    def dw_dve_single(w):
        # one 4-row window on DVE (single accumulator)
        a = accv[:, dve_w_idx[w] * WPX : (dve_w_idx[w] + 1) * WPX]
        nc.gpsimd.tensor_scalar_mul(a, win_ap(w, 0, 0), kcols[:, 0:1])
        for t in range(1, KK * KK):
            nc.vector.scalar_tensor_tensor(
                a, win_ap(w, *TAPS[t]), kcols[:, t : t + 1], a,
                op0=mybir.AluOpType.mult, op1=mybir.AluOpType.add,
            )
        nc.scalar.activation(D[:, w * WPX : (w + 1) * WPX], a, PRELU, alpha=0.1)

    assert all(2 * (w // 2) in dve_w_idx and 2 * (w // 2) + 1 in dve_w_idx
               for w in DVE_WINDOWS if w not in DVE_EXTRA), \
        "DVE pair windows must come in full pairs"

    def conv1x1_pair(u, evac_dve=False):
        O2 = po.tile([128, 2 * WPX], f32, tag="po")
        for half in range(2):
            w = 2 * u + half
            nc.tensor.matmul(
                O2[:, half * WPX : (half + 1) * WPX],
                lhsT=wcb[:, :],
                rhs=D[:, w * WPX : (w + 1) * WPX],
                start=True, stop=True,
                skip_group_check=True,
            )
        d2 = D2[:, 2 * u * WPX : (2 * u + 2) * WPX]
        if evac_dve:
            nc.vector.tensor_scalar_add(d2, O2[:, :], bc2)
        else:
            nc.scalar.activation(d2, O2[:, :], PRELU,
                                 bias=bc2, scale=1.0, alpha=1.0)

    def conv1x1_single(w, evac_dve=False):
        O1 = po.tile([128, WPX], f32, tag="po", name=f"po{w}")
        nc.tensor.matmul(
            O1[:, :], lhsT=wcb[:, :], rhs=D[:, w * WPX : (w + 1) * WPX],
            start=True, stop=True, skip_group_check=True,
        )
        d2 = D2[:, w * WPX : (w + 1) * WPX]
        if evac_dve:
            nc.vector.tensor_scalar_add(d2, O1[:, :], bc2)
        else:
            nc.scalar.activation(d2, O1[:, :], PRELU,
                                 bias=bc2, scale=1.0, alpha=1.0)

    # warm-up matmuls: ramp the PE p-state while the x DMA streams in.
    # Reads the (memset) top halo row; result discarded.
    warm = po.tile([128, 128], f32, tag="po", name="warm")
    for _ in range(int(os.environ.get("DA_WARM", "1"))):
        nc.tensor.matmul(warm[:, :], lhsT=xs[:, 0:128], rhs=xs[:, 0:128],
                         start=True, stop=True, skip_group_check=True)
    # touch ScalarE once so its Prelu table load happens during the prologue
    # instead of stalling the first PSUM evacuation
    wsc = const.tile([128, 1], f32)
    nc.scalar.activation(wsc[:, :], warm[:, 0:1], PRELU, alpha=0.1)

    for u in range(NU):
        if u == 0:
            dw_pe_single(0)
            dw_pe_single(1)
        elif u == NU - 1:
            for q, (ro, nr) in enumerate(TAIL_PIECES):
                dw_pe_half(2 * u, q, ro, nr)
        elif 2 * u in dve_w_idx:
            dw_dve_pair(u, merge=(u in MERGE_UNITS))
        elif 2 * u in [w - (w % 2) for w in DVE_EXTRA]:
            dw_dve_single(2 * u)
            dw_pe_single(2 * u + 1)
        elif u == 11 and SLIVER:
            dw_pe_single(22)
            dw_dve_sliver(23)
            dw_pe_rest(23)
        else:
            dw_pe_pair(u)
        if u == NU - 1:
            OQA = po.tile([128, WPX], f32, tag="po", name="oqa")
            OQB = po.tile([128, WPX], f32, tag="po", name="oqb")
            for q, (ro, nr) in enumerate(TAIL_PIECES):
                dst = OQA if ro < WR else OQB
                doff = (ro % WR) * W
                nc.tensor.matmul(
                    dst[:, doff : doff + nr * W], lhsT=wcb[:, :],
                    rhs=D[:, 2 * u * WPX + ro * W : 2 * u * WPX + (ro + nr) * W],
                    start=True, stop=True, skip_group_check=True,
                )
            nc.scalar.activation(
                D2[:, 2 * u * WPX : (2 * u + 1) * WPX], OQA[:, :], PRELU,
                bias=bc2, scale=1.0, alpha=1.0,
            )
            nc.vector.tensor_scalar_add(
                D2[:, (2 * u + 1) * WPX : (2 * u + 2) * WPX], OQB[:, :], bc2,
            )
            o = (2 * u - 2) * WPX
            nc.sync.dma_start(out_d[:, o : o + 2 * WPX], D2[:, o : o + 2 * WPX])
            o += 2 * WPX
            nc.sync.dma_start(out_d[:, o : o + WPX], D2[:, o : o + WPX])
            o += WPX
            nc.sync.dma_start(out_d[:, o : o + WPX], D2[:, o : o + WPX])
        else:
            conv1x1_pair(u, evac_dve=(u == NU - 2))
            if u % 2 == 1:
                o = (u - 1) * 2 * WPX
                nc.sync.dma_start(out_d[:, o : o + 4 * WPX], D2[:, o : o + 4 * WPX])


# ---------------------------------------------------------------------------
# host-side entry point
# ---------------------------------------------------------------------------

_PROGRAM_CACHE: dict[str, bass.Bass] = {}


def _get_program() -> bass.Bass:
    if "p" not in _PROGRAM_CACHE:
        _PROGRAM_CACHE["p"] = build_program()
    return _PROGRAM_CACHE["p"]


def _host_prep(inputs: dict):
    x = np.asarray(inputs["x"], dtype=np.float32)
    d = np.asarray(inputs["d"], dtype=np.float32)
    Wk1 = np.asarray(inputs["Wk1"], dtype=np.float32)
    Wk2 = np.asarray(inputs["Wk2"], dtype=np.float32)
    Wc = np.asarray(inputs["Wc"], dtype=np.float32)
    bc = np.asarray(inputs["bc"], dtype=np.float32)

    # kernel-generating MLP on the host (tiny): kern[b, c, t]
    hid = d @ Wk1.T
    hid = np.where(hid >= 0, hid, 0.1 * hid)
    kern = (hid @ Wk2.T).reshape(-1, C, KK * KK)
    bc2 = np.concatenate([bc, bc]).reshape(2 * C, 1)

    wcb = np.zeros((2 * C, 2 * C), dtype=np.float32)
    wcb[0:C, 0:C] = Wc.T
    wcb[C:, C:] = Wc.T
    wcb = wcb.astype(np.float16)

    xcast = x.astype(np.float16)

    in_maps = []
    for i in range(NCORES):
        xi = np.ascontiguousarray(xcast[S * i : S * (i + 1)].reshape(S * C, H * W))
        kc = kern[S * i : S * (i + 1)].reshape(2 * C, KK * KK)  # (s*64+c, t)
        dg = np.zeros((2 * C, KK * KK, 2 * C), dtype=np.float32)
        idx = np.arange(2 * C)
        dg[idx, :, idx] = kc
        kd = np.concatenate(
            [bc2, kc, dg.reshape(2 * C, KK * KK * 2 * C)], axis=1
        ).astype(np.float16)
        kd = np.ascontiguousarray(kd)
        in_maps.append({"x": xi, "kd": kd, "wcb": wcb})
    return in_maps


def run_on_hw(inputs: dict, **kwargs):
    """Run the SPMD kernel on 8 NeuronCores; returns (output, BassKernelResults)."""
    from concourse.bass_utils import run_bass_kernel_spmd

    nc = _get_program()
    in_maps = _host_prep(inputs)
    res = run_bass_kernel_spmd(nc, in_maps, core_ids=list(range(NCORES)), **kwargs)
    outs = res.results
    B = S * NCORES
    out = np.empty((B, C, H, W), dtype=np.float32)
    for i in range(NCORES):
        out[S * i : S * (i + 1)] = (
            outs[i]["out"].view(np.float16).astype(np.float32).reshape(S, C, H, W)
        )
    return out, res


def kernel(**inputs) -> np.ndarray:
    out, _ = run_on_hw(inputs)
    return out


if __name__ == "__main__":
    nc = build_program()
    print("program built OK")


# revision 42
# speedup vs baseline: 1.0135x; 1.0110x over previous
"""Trainium2 Bass kernel for nn_DA_conv: per-sample dynamic depthwise 3x3 conv
(+LeakyReLU) followed by a 1x1 pointwise conv, with the 3x3 kernels produced by
a small per-sample MLP.

Strategy (8 NeuronCores, pure batch data-parallel, 2 samples per core):
  - The tiny kernel-generating MLP runs on the host; the device receives the
    per-(sample,channel) tap scalars and prebuilt 128x128 block-diagonal fp16
    tap matrices in a single small DMA.
  - SBUF partition p = (sample s = p//64, channel c = p%64); fp16 feature map
    resident in SBUF with zero-padded borders (row stride 132, halo rows).
  - Depthwise 3x3 conv split spatially across engines, 4-row windows (512 px
    = one PSUM bank) as the unit:
      * PE windows (22): 9 PSUM-accumulating 128-partition block-diagonal
        fp16 matmuls per window (both samples in one matmul = 1 cycle/row).
      * DVE pair-units (5, interleaved ~1:2 with PE units to track the x-DMA
        arrival frontier): 8-row in-place MAC chains (scalar_tensor_tensor)
        into f32 SBUF accumulators. Pairs split into two partials - products
        on GPSIMD + ScalarE, 7 MACs on VectorE, merge-add on GPSIMD - so all
        four engines contribute. The last DVE unit stays single-accumulator
        and lrelu-evacuates itself (max(x, 0.1x)) so the drain never queues
        on ScalarE.
  - LeakyReLU evacuation (PSUM/SBUF acc -> fp16 D) on ScalarE via Prelu; the
    Prelu activation table is preloaded at t=0 via a dummy op so the first
    evacuation does not stall on the 1.3us table load.
  - 1x1 conv = one 128x128 block-diag fp16 matmul per window; bias added
    during the PSUM->SBUF evac on ScalarE via Prelu(alpha=1) with a bias AP.
  - A single early warm-up matmul starts the PE p-state ramp clock during
    the DMA prologue, so real matmuls run at full rate from the start.
  - Output staged as fp16 (host upcasts); the final pair unit runs as 2-row
    windows with evacuations split across ScalarE/VectorE and fine-grained
    output DMAs to minimize the drain chain.
"""

import os
import sys

sys.path.insert(0, "/opt/trn_rl_repo")

from contextlib import ExitStack

import numpy as np

import concourse.bacc as bacc
import concourse.bass as bass
import concourse.mybir as mybir
import concourse.tile as tile

S = 2            # samples per core
C = 64           # channels
H = W = 128      # spatial
KK = 3           # conv kernel size
NCORES = 8
RS = W + 4       # padded row stride (132 fp16 -> 264B; pads at cols 0,129..131)
RP = H + 2       # padded row count (top/bottom halo)
XFREE = RP * RS
WR = 4           # image rows per window
NWIN = H // WR   # 32 windows
NU = NWIN // 2   # 16 pair units
WPX = WR * W     # 512 px per window = one PSUM bank of f32

f32 = mybir.dt.float32
fp16 = mybir.dt.float16
i32 = mybir.dt.int32

PRELU = mybir.ActivationFunctionType.Prelu
TAPS = [(di, dj) for di in range(KK) for dj in range(KK)]  # t = di*3 + dj

# Pair units owned by the DVE MAC path, interleaved ~1:2 with PE units so
# both engines consume windows near the x-DMA arrival frontier; the last
# units stay on PE so the trailing 1x1 matmuls never wait on VectorE.
N_DVE_UNITS = int(os.environ.get("DA_NDVE", "5"))
_DVE_POS = {
    "a": [1, 4, 7, 10, 13],
    "b": [1, 4, 6, 9, 13],
    "c": [2, 5, 8, 11, 13],
    "d": [1, 3, 6, 9, 12],
}
DVE_UNITS = (_DVE_POS[os.environ.get("DA_POS", "a")] if N_DVE_UNITS == 5
             else [1, 4, 5, 7, 10, 13][:N_DVE_UNITS])
MERGE_UNITS = (set(DVE_UNITS[:-1])
               if os.environ.get("DA_MERGE", "1") == "1" else set())
DVE_EXTRA = [22] if os.environ.get("DA_EXTRA", "0") == "1" else []
SLIVER = os.environ.get("DA_SLIVER", "0") == "1"
# final pair processed in tapering pieces (row offset, nrows) over its 8 rows
TAIL_PIECES = [(0, 2), (2, 2), (4, 2), (6, 2)]
DVE_WINDOWS = sorted(
    [w for u in DVE_UNITS for w in (2 * u, 2 * u + 1)] + DVE_EXTRA
)


def build_program() -> bass.Bass:
    nc = bacc.Bacc("TRN2", target_bir_lowering=False, debug=False)

    x_d = nc.dram_tensor("x", [S * C, XFREE], fp16, kind="ExternalInput").ap()
    # host-computed fp16 small weights: [bc (1) | kcols (9) | diag (9*128)]
    kd_d = nc.dram_tensor("kd", [S * C, 10 + KK * KK * S * C], fp16,
                          kind="ExternalInput").ap()
    # block-diagonal duplicated 1x1 weights, fp16
    wcb_d = nc.dram_tensor("wcb", [S * C, S * C], fp16, kind="ExternalInput").ap()
    out_d = nc.dram_tensor("out", [S * C, H * W], fp16, kind="ExternalOutput").ap()

    with tile.TileContext(nc) as tc, ExitStack() as ctx:
        _body(ctx, tc, x_d, kd_d, wcb_d, out_d)
    nc.compile()
    return nc


def _body(ctx, tc, x_d, kd_d, wcb_d, out_d):
    nc = tc.nc
    const = ctx.enter_context(tc.tile_pool(name="const", bufs=1))
    xpool = ctx.enter_context(tc.tile_pool(name="xs", bufs=1))
    dpool = ctx.enter_context(tc.tile_pool(name="dd", bufs=1))
    pdw = ctx.enter_context(tc.tile_pool(name="pdw", bufs=2, space="PSUM"))
    po = ctx.enter_context(tc.tile_pool(name="po", bufs=2, space="PSUM"))

    # ---------------- weight loads ----------------
    kd = const.tile([2 * C, 10 + KK * KK * 2 * C], fp16)
    diag = kd[:, 10:]
    kbf = const.tile([2 * C, 10], f32)
    bc2 = kbf[:, 0:1]
    kcols = kbf[:, 1:10]

    # ---------------- resident padded feature map ----------------
    # The host ships x pre-padded (halo rows + pad columns baked in), so every
    # chunk is a fully contiguous [128, n*RS] transfer with ~4KB descriptors
    # (2x the bus efficiency of 256B padded-row segments).
    xs = xpool.tile([2 * C, XFREE], fp16)

    def dma_chunk(pr0, prn):
        nc.sync.dma_start(
            xs[:, pr0 * RS : (pr0 + prn) * RS],
            x_d[:, pr0 * RS : (pr0 + prn) * RS],
        )

    # padded-row chunks: [0,6) covers halo+img rows 0-4 (window 0), etc.;
    # the last chunk ends at RP covering the bottom halo row.
    dma_chunk(0, 6)
    nc.sync.dma_start(kd[:, :], kd_d)
    # per-partition scalars must be f32 for DVE/GPSIMD ops
    nc.vector.tensor_copy(kbf[:, :], kd[:, 0:10])
    dma_chunk(6, 11)
    dma_chunk(17, 8)
    dma_chunk(25, 8)
    wcb = const.tile([2 * C, 2 * C], fp16)
    nc.sync.dma_start(wcb[:, :], wcb_d)
    r0 = 33
    for cr in [12, 12, 12, 12, 16, 16, 17]:
        dma_chunk(r0, cr)
        r0 += cr

    # ---------------- main loop ----------------
    xrows = xs[:, :].rearrange("p (r w) -> p r w", w=RS)
    D = dpool.tile([2 * C, H * W], fp16)    # lrelu(dw), 1x1 rhs
    D2 = dpool.tile([2 * C, H * W], fp16)   # 1x1 out + bias, DMA staging
    n_dve_w = len(DVE_WINDOWS)
    accv = (dpool.tile([2 * C, n_dve_w * WPX + WPX // 2, ], f32, name="accv")
            if n_dve_w else None)
    accbp = ctx.enter_context(tc.tile_pool(name="accb", bufs=2))
    dve_w_idx = {w: i for i, w in enumerate(DVE_WINDOWS)}

    def win_ap(w, di, dj):
        # rhs/in0 window for tap (di, dj) over image rows 4w..4w+3
        return xrows[:, WR * w + di : WR * w + di + WR, dj : dj + W]

    def dw_pe(w, dst):
        for t, (di, dj) in enumerate(TAPS):
            nc.tensor.matmul(
                dst,
                lhsT=diag[:, t * 128 : (t + 1) * 128],
                rhs=win_ap(w, di, dj),
                start=(t == 0), stop=(t == KK * KK - 1),
                skip_group_check=True,
            )

    def dw_pe_pair(u):
        P2 = pdw.tile([128, 2 * WPX], f32, tag="pdw")
        for half in range(2):
            dw_pe(2 * u + half, P2[:, half * WPX : (half + 1) * WPX])
        nc.scalar.activation(
            D[:, 2 * u * WPX : (2 * u + 2) * WPX], P2[:, :], PRELU, alpha=0.1
        )

    def dw_pe_half(w0, q, ro, nr):
        # nr-row (nr*128 px) piece of the final pair, at row offset ro
        px = nr * W
        P1 = pdw.tile([128, px], f32, tag="pdw", name=f"ph{q}")
        for t, (di, dj) in enumerate(TAPS):
            nc.tensor.matmul(
                P1[:, :],
                lhsT=diag[:, t * 128 : (t + 1) * 128],
                rhs=xrows[:, WR * w0 + ro + di : WR * w0 + ro + di + nr,
                          dj : dj + W],
                start=(t == 0), stop=(t == KK * KK - 1),
                skip_group_check=True,
            )
        o = w0 * WPX + ro * W
        nc.scalar.activation(D[:, o : o + px], P1[:, :], PRELU, alpha=0.1)

    def dw_pe_single(w, evac_dve=False):
        P1 = pdw.tile([128, WPX], f32, tag="pdw", name=f"ps{w}")
        dw_pe(w, P1[:, :])
        dd = D[:, w * WPX : (w + 1) * WPX]
        if evac_dve:
            # lrelu(x) = max(x, 0.1x) as a single DVE op
            nc.vector.scalar_tensor_tensor(
                dd, P1[:, :], 0.1, P1[:, :],
                op0=mybir.AluOpType.mult, op1=mybir.AluOpType.max,
            )
        else:
            nc.scalar.activation(dd, P1[:, :], PRELU, alpha=0.1)

    def dw_dve_pair(u, merge=False):
        # 8-row (1024 px) in-place DVE MAC chains over both windows of the
        # unit at once; tap-0 product on GPSIMD. With merge=True, taps split
        # into two partial accumulators (second product on GPSIMD, merge-add
        # on GPSIMD) so VectorE only runs 7 of the 9 taps.
        def pwin(di, dj):
            return xrows[:, 8 * u + di : 8 * u + di + 8, dj : dj + W]

        base = dve_w_idx[2 * u] * WPX
        a = accv[:, base : base + 2 * WPX]
        if merge:
            # two partials: tap-0 product on GPSIMD, tap-4 product on ScalarE,
            # 3+4 MACs on VectorE, merge-add on GPSIMD
            nc.gpsimd.tensor_scalar_mul(a, pwin(0, 0), kcols[:, 0:1])
            b = accbp.tile([2 * C, 2 * WPX], f32, tag="accB", name=f"ab{u}")
            nc.scalar.mul(b[:, :], pwin(*TAPS[4]), kcols[:, 4:5])
            for t in (1, 2, 3):
                nc.vector.scalar_tensor_tensor(
                    a, pwin(*TAPS[t]), kcols[:, t : t + 1], a,
                    op0=mybir.AluOpType.mult, op1=mybir.AluOpType.add,
                )
            for t in (5, 6, 7, 8):
                nc.vector.scalar_tensor_tensor(
                    b[:, :], pwin(*TAPS[t]), kcols[:, t : t + 1], b[:, :],
                    op0=mybir.AluOpType.mult, op1=mybir.AluOpType.add,
                )
            nc.gpsimd.tensor_tensor(a, a, b[:, :], mybir.AluOpType.add)
        else:
            nc.gpsimd.tensor_scalar_mul(a, pwin(0, 0), kcols[:, 0:1])
            for t in range(1, KK * KK):
                nc.vector.scalar_tensor_tensor(
                    a, pwin(*TAPS[t]), kcols[:, t : t + 1], a,
                    op0=mybir.AluOpType.mult, op1=mybir.AluOpType.add,
                )
        dd = D[:, 2 * u * WPX : (2 * u + 2) * WPX]
        if u == DVE_UNITS[-1]:
            # acc is SBUF: lrelu(x) = max(x, 0.1x) directly on the (by now
            # otherwise idle) VectorE, keeping the drain off ScalarE's queue
            nc.vector.scalar_tensor_tensor(
                dd, a, 0.1, a, op0=mybir.AluOpType.mult,
                op1=mybir.AluOpType.max,
            )
        else:
            nc.scalar.activation(dd, a, PRELU, alpha=0.1)

    def dw_dve_sliver(w):
        # first 2 rows (256 px) of window w on DVE, into the spare acc slot
        HPX = WPX // 2
        a = accv[:, len(DVE_WINDOWS) * WPX : len(DVE_WINDOWS) * WPX + HPX]
        win = lambda di, dj: xrows[:, WR * w + di : WR * w + di + 2, dj : dj + W]
        nc.gpsimd.tensor_scalar_mul(a, win(0, 0), kcols[:, 0:1])
        for t in range(1, KK * KK):
            nc.vector.scalar_tensor_tensor(
                a, win(*TAPS[t]), kcols[:, t : t + 1], a,
                op0=mybir.AluOpType.mult, op1=mybir.AluOpType.add,
            )
        nc.scalar.activation(
            D[:, w * WPX : w * WPX + HPX], a, PRELU, alpha=0.1
        )

    def dw_pe_rest(w):
        # rows 2..3 (256 px) of window w on PE
        HPX = WPX // 2
        P1 = pdw.tile([128, HPX], f32, tag="pdw", name=f"pr{w}")
        for t, (di, dj) in enumerate(TAPS):
            nc.tensor.matmul(
                P1[:, :],
                lhsT=diag[:, t * 128 : (t + 1) * 128],
                rhs=xrows[:, WR * w + 2 + di : WR * w + 2 + di + 2, dj : dj + W],
                start=(t == 0), stop=(t == KK * KK - 1),
                skip_group_check=True,
            )
        nc.scalar.activation(
            D[:, w * WPX + HPX : (w + 1) * WPX], P1[:, :], PRELU, alpha=0.1
        )

    def dw_dve_single(w):
        # one 4-row window on DVE (single accumulator)
        a = accv[:, dve_w_idx[w] * WPX : (dve_w_idx[w] + 1) * WPX]
        nc.gpsimd.tensor_scalar_mul(a, win_ap(w, 0, 0), kcols[:, 0:1])
        for t in range(1, KK * KK):
            nc.vector.scalar_tensor_tensor(
                a, win_ap(w, *TAPS[t]), kcols[:, t : t + 1], a,
                op0=mybir.AluOpType.mult, op1=mybir.AluOpType.add,
            )
        nc.scalar.activation(D[:, w * WPX : (w + 1) * WPX], a, PRELU, alpha=0.1)

    assert all(2 * (w // 2) in dve_w_idx and 2 * (w // 2) + 1 in dve_w_idx
               for w in DVE_WINDOWS if w not in DVE_EXTRA), \
        "DVE pair windows must come in full pairs"

    def conv1x1_pair(u, evac_dve=False):
        O2 = po.tile([128, 2 * WPX], f32, tag="po")
        for half in range(2):
            w = 2 * u + half
            nc.tensor.matmul(
                O2[:, half * WPX : (half + 1) * WPX],
                lhsT=wcb[:, :],
                rhs=D[:, w * WPX : (w + 1) * WPX],
                start=True, stop=True,
                skip_group_check=True,
            )
        d2 = D2[:, 2 * u * WPX : (2 * u + 2) * WPX]
        if evac_dve:
            nc.vector.tensor_scalar_add(d2, O2[:, :], bc2)
        else:
            nc.scalar.activation(d2, O2[:, :], PRELU,
                                 bias=bc2, scale=1.0, alpha=1.0)

    def conv1x1_single(w, evac_dve=False):
        O1 = po.tile([128, WPX], f32, tag="po", name=f"po{w}")
        nc.tensor.matmul(
            O1[:, :], lhsT=wcb[:, :], rhs=D[:, w * WPX : (w + 1) * WPX],
            start=True, stop=True, skip_group_check=True,
        )
        d2 = D2[:, w * WPX : (w + 1) * WPX]
        if evac_dve:
            nc.vector.tensor_scalar_add(d2, O1[:, :], bc2)
        else:
            nc.scalar.activation(d2, O1[:, :], PRELU,
                                 bias=bc2, scale=1.0, alpha=1.0)

    # warm-up matmuls: ramp the PE p-state while the x DMA streams in.
    # Reads the (memset) top halo row; result discarded.
    warm = po.tile([128, 128], f32, tag="po", name="warm")
    for _ in range(int(os.environ.get("DA_WARM", "1"))):
        nc.tensor.matmul(warm[:, :], lhsT=xs[:, 0:128], rhs=xs[:, 0:128],
                         start=True, stop=True, skip_group_check=True)
    # touch ScalarE once so its Prelu table load happens during the prologue
    # instead of stalling the first PSUM evacuation
    wsc = const.tile([128, 1], f32)
    nc.scalar.activation(wsc[:, :], warm[:, 0:1], PRELU, alpha=0.1)

    for u in range(NU):
        if u == 0:
            dw_pe_single(0)
            dw_pe_single(1)
        elif u == NU - 1:
            for q, (ro, nr) in enumerate(TAIL_PIECES):
                dw_pe_half(2 * u, q, ro, nr)
        elif 2 * u in dve_w_idx:
            dw_dve_pair(u, merge=(u in MERGE_UNITS))
        elif 2 * u in [w - (w % 2) for w in DVE_EXTRA]:
            dw_dve_single(2 * u)
            dw_pe_single(2 * u + 1)
        elif u == 11 and SLIVER:
            dw_pe_single(22)
            dw_dve_sliver(23)
            dw_pe_rest(23)
        else:
            dw_pe_pair(u)
        if u == NU - 1:
            OQA = po.tile([128, WPX], f32, tag="po", name="oqa")
            OQB = po.tile([128, WPX], f32, tag="po", name="oqb")
            for q, (ro, nr) in enumerate(TAIL_PIECES):
                dst = OQA if ro < WR else OQB
                doff = (ro % WR) * W
                nc.tensor.matmul(
                    dst[:, doff : doff + nr * W], lhsT=wcb[:, :],
                    rhs=D[:, 2 * u * WPX + ro * W : 2 * u * WPX + (ro + nr) * W],
                    start=True, stop=True, skip_group_check=True,
                )
            nc.scalar.activation(
                D2[:, 2 * u * WPX : (2 * u + 1) * WPX], OQA[:, :], PRELU,
                bias=bc2, scale=1.0, alpha=1.0,
            )
            nc.vector.tensor_scalar_add(
                D2[:, (2 * u + 1) * WPX : (2 * u + 2) * WPX], OQB[:, :], bc2,
            )
            o = (2 * u - 2) * WPX
            nc.sync.dma_start(out_d[:, o : o + 2 * WPX], D2[:, o : o + 2 * WPX])
            o += 2 * WPX
            nc.sync.dma_start(out_d[:, o : o + WPX], D2[:, o : o + WPX])
            o += WPX
            nc.sync.dma_start(out_d[:, o : o + WPX], D2[:, o : o + WPX])
        else:
            conv1x1_pair(u, evac_dve=(u == NU - 2))
            if u % 2 == 1:
                o = (u - 1) * 2 * WPX
                nc.sync.dma_start(out_d[:, o : o + 4 * WPX], D2[:, o : o + 4 * WPX])


# ---------------------------------------------------------------------------
# host-side entry point
# ---------------------------------------------------------------------------

_PROGRAM_CACHE: dict[str, bass.Bass] = {}


def _get_program() -> bass.Bass:
    if "p" not in _PROGRAM_CACHE:
        _PROGRAM_CACHE["p"] = build_program()
    return _PROGRAM_CACHE["p"]


B_TOTAL = S * NCORES


def _host_prep(inputs: dict):
    x = np.asarray(inputs["x"], dtype=np.float32)
    d = np.asarray(inputs["d"], dtype=np.float32)
    Wk1 = np.asarray(inputs["Wk1"], dtype=np.float32)
    Wk2 = np.asarray(inputs["Wk2"], dtype=np.float32)
    Wc = np.asarray(inputs["Wc"], dtype=np.float32)
    bc = np.asarray(inputs["bc"], dtype=np.float32)

    # kernel-generating MLP on the host (tiny): kern[b, c, t]
    hid = d @ Wk1.T
    hid = np.where(hid >= 0, hid, 0.1 * hid)
    kern = (hid @ Wk2.T).reshape(-1, C, KK * KK)
    bc2 = np.concatenate([bc, bc]).reshape(2 * C, 1)

    wcb = np.zeros((2 * C, 2 * C), dtype=np.float32)
    wcb[0:C, 0:C] = Wc.T
    wcb[C:, C:] = Wc.T
    wcb = wcb.astype(np.float16)

    xpad = np.zeros((B_TOTAL, C, RP, RS), dtype=np.float16)
    xpad[:, :, 1 : H + 1, 1 : W + 1] = x
    xpad = xpad.reshape(B_TOTAL, C, RP * RS)

    in_maps = []
    for i in range(NCORES):
        xi = np.ascontiguousarray(
            xpad[S * i : S * (i + 1)].reshape(S * C, RP * RS)
        )
        kc = kern[S * i : S * (i + 1)].reshape(2 * C, KK * KK)  # (s*64+c, t)
        dg = np.zeros((2 * C, KK * KK, 2 * C), dtype=np.float32)
        idx = np.arange(2 * C)
        dg[idx, :, idx] = kc
        kd = np.concatenate(
            [bc2, kc, dg.reshape(2 * C, KK * KK * 2 * C)], axis=1
        ).astype(np.float16)
        kd = np.ascontiguousarray(kd)
        in_maps.append({"x": xi, "kd": kd, "wcb": wcb})
    return in_maps


def run_on_hw(inputs: dict, **kwargs):
    """Run the SPMD kernel on 8 NeuronCores; returns (output, BassKernelResults)."""
    from concourse.bass_utils import run_bass_kernel_spmd

    nc = _get_program()
    in_maps = _host_prep(inputs)
    res = run_bass_kernel_spmd(nc, in_maps, core_ids=list(range(NCORES)), **kwargs)
    outs = res.results
    B = S * NCORES
    out = np.empty((B, C, H, W), dtype=np.float32)
    for i in range(NCORES):
        out[S * i : S * (i + 1)] = (
            outs[i]["out"].view(np.float16).astype(np.float32).reshape(S, C, H, W)
        )
    return out, res


def kernel(**inputs) -> np.ndarray:
    out, _ = run_on_hw(inputs)
    return out


if __name__ == "__main__":
    nc = build_program()
    print("program built OK")
